# revision 8
# baseline (speedup 1.0000x reference)
"""Bass/Tile kernel for nn_Head (softmax-first attention with post-softmax
strict-upper causal mask), SPMD over 8 TRN2 NeuronCores.

  q = x @ Wq; k = y @ Wk; v = y @ Wv        (B=4, N=M=4096, C=1024, D=128)
  a = softmax(q k^T / sqrt(D))              (full-row softmax)
  a = triu(a, k=1)                          (post-softmax mask, keeps j > i)
  out = a @ v

Sharding: core (b, h) = (core//2, core%2) handles batch b, row-blocks
2t+h (t=0..15) of 128 rows each (interleaved for AV load balance).
"""
import sys
sys.path.insert(0, '/opt/trn_rl_repo')

from contextlib import ExitStack

import numpy as np
import ml_dtypes

import concourse.bass as bass
import concourse.bacc as bacc
import concourse.tile as tile
from concourse import mybir
from concourse.bass_utils import run_bass_kernel_spmd
from concourse.masks import make_identity

F32 = mybir.dt.float32
BF16 = mybir.dt.bfloat16
NPBF16 = ml_dtypes.bfloat16

B, N, M, C, D = 4, 4096, 4096, 1024, 128
NCORES = 8
NLOC = N // 2              # 2048 rows per core
NBLK = NLOC // 128         # 16 i-blocks per core
JCH = M // 128             # 32 j-chunks
CCH = C // 128             # 8 contraction chunks
SCALE = 1.0 / np.sqrt(np.float32(D))

_CACHE = {}
TRACE = False
TRACE_DIR = "/tmp/attn_trace"


def build_nc():
    nc = bacc.Bacc("TRN2", target_bir_lowering=False, debug=False,
                   num_devices=NCORES)
    xsT_d = nc.dram_tensor("xsT", [C, NLOC], BF16, kind="ExternalInput").ap()
    yT_d = nc.dram_tensor("yT", [C, M], BF16, kind="ExternalInput").ap()
    wq_d = nc.dram_tensor("wq", [C, D], BF16, kind="ExternalInput").ap()
    wk_d = nc.dram_tensor("wk", [C, D], BF16, kind="ExternalInput").ap()
    wv_d = nc.dram_tensor("wv", [C, D], BF16, kind="ExternalInput").ap()
    dmask_d = nc.dram_tensor("dmask", [128, 256], BF16, kind="ExternalInput").ap()
    out_d = nc.dram_tensor("out", [NLOC, D], F32, kind="ExternalOutput").ap()

    with tile.TileContext(nc) as tc:
        with ExitStack() as ctx:
            const = ctx.enter_context(tc.tile_pool(name="const", bufs=1))
            stage = ctx.enter_context(tc.tile_pool(name="stage", bufs=3))
            big = ctx.enter_context(tc.tile_pool(name="big", bufs=1))
            epool = ctx.enter_context(tc.tile_pool(name="epool", bufs=2))
            atsb = ctx.enter_context(tc.tile_pool(name="atsb", bufs=3))
            small = ctx.enter_context(tc.tile_pool(name="small", bufs=4))
            outp = ctx.enter_context(tc.tile_pool(name="outp", bufs=3))
            # PSUM pools: S 4 banks + at 2 + ps_small 2 = 8 banks
            spsum = ctx.enter_context(tc.tile_pool(name="spsum", bufs=1, space="PSUM"))
            atpsum = ctx.enter_context(tc.tile_pool(name="atpsum", bufs=2, space="PSUM"))
            pssm = ctx.enter_context(tc.tile_pool(name="pssm", bufs=2, space="PSUM"))

            # ---- constants ----
            wq_sb = const.tile([128, CCH, D], BF16)
            wk_sb = const.tile([128, CCH, D], BF16)
            wv_sb = const.tile([128, CCH, D], BF16)
            nc.sync.dma_start(out=wq_sb, in_=wq_d.rearrange("(c p) d -> p c d", p=128))
            nc.sync.dma_start(out=wk_sb, in_=wk_d.rearrange("(c p) d -> p c d", p=128))
            nc.sync.dma_start(out=wv_sb, in_=wv_d.rearrange("(c p) d -> p c d", p=128))
            dmask_sb = const.tile([128, 256], BF16)
            nc.sync.dma_start(out=dmask_sb, in_=dmask_d)
            ident = const.tile([128, 128], BF16)
            make_identity(nc, ident)

            # ---- resident tensors ----
            kT_sb = big.tile([128, M], BF16)          # k^T [d, j]
            v_sb = big.tile([128, JCH, D], BF16)      # v [j-in-chunk, chunk, d]
            qT_sb = big.tile([128, NLOC], BF16)       # q^T [d, i]

            # ---- phase 1: k^T and v from y ----
            yT_view = yT_d.rearrange("(c p) m -> p c m", p=128)
            xsT_view = xsT_d.rearrange("(c p) n -> p c n", p=128)
            for jt in range(M // 512):
                yT = stage.tile([128, CCH, 512], BF16, tag="xyT")
                nc.sync.dma_start(out=yT,
                                  in_=yT_view[:, :, jt * 512:(jt + 1) * 512])
                kT_ps = pssm.tile([128, 512], F32, tag="ps_small")
                for c in range(CCH):
                    nc.tensor.matmul(kT_ps, wk_sb[:, c, :], yT[:, c, :],
                                     start=(c == 0), stop=(c == CCH - 1))
                nc.any.tensor_copy(kT_sb[:, jt * 512:(jt + 1) * 512], kT_ps)
                v_ps = pssm.tile([128, 4, 128], F32, tag="ps_small")
                for b4 in range(4):
                    for c in range(CCH):
                        nc.tensor.matmul(v_ps[:, b4, :],
                                         yT[:, c, b4 * 128:(b4 + 1) * 128],
                                         wv_sb[:, c, :],
                                         start=(c == 0), stop=(c == CCH - 1))
                nc.any.tensor_copy(v_sb[:, 4 * jt:4 * jt + 4, :], v_ps)

            # ---- phase 1.5: q^T from xs ----
            for tau in range(NLOC // 512):
                xT = stage.tile([128, CCH, 512], BF16, tag="xyT")
                nc.sync.dma_start(out=xT,
                                  in_=xsT_view[:, :, tau * 512:(tau + 1) * 512])
                qT_ps = pssm.tile([128, 512], F32, tag="ps_small")
                for c in range(CCH):
                    nc.tensor.matmul(qT_ps, wq_sb[:, c, :], xT[:, c, :],
                                     start=(c == 0), stop=(c == CCH - 1))
                nc.any.tensor_copy(qT_sb[:, tau * 512:(tau + 1) * 512], qT_ps)

            # ---- phase 2: attention per i-block ----
            for t in range(NBLK):
                lhs_q = qT_sb[:, t * 128:(t + 1) * 128]
                E = epool.tile([128, M], BF16, tag="E")
                den = small.tile([128, 2], F32, tag="den")
                for half in range(2):
                    S_ps = spsum.tile([128, 4, 512], F32, tag="S")
                    for jj in range(4):
                        jt = half * 4 + jj
                        nc.tensor.matmul(S_ps[:, jj, :], lhs_q,
                                         kT_sb[:, jt * 512:(jt + 1) * 512],
                                         start=True, stop=True)
                    nc.scalar.activation(
                        E[:, half * 2048:(half + 1) * 2048],
                        S_ps.rearrange("p a b -> p (a b)"),
                        mybir.ActivationFunctionType.Exp,
                        scale=float(SCALE),
                        accum_out=den[:, half:half + 1])
                dsum = small.tile([128, 1], F32, tag="dsum")
                rden = small.tile([128, 1], F32, tag="rden")
                nc.vector.tensor_add(dsum, den[:, 0:1], den[:, 1:2])
                nc.vector.reciprocal(rden, dsum)
                # mask the two diagonal chunks (2t: zero/strict-upper, 2t+1)
                nc.vector.tensor_mul(E[:, 256 * t:256 * t + 256],
                                     E[:, 256 * t:256 * t + 256], dmask_sb)
                # A^T transposes + AV accumulation over kept chunks
                kept = list(range(2 * t, JCH))
                av_ps = pssm.tile([128, 128], F32, tag="ps_small")
                for g0 in range(0, len(kept), 8):
                    grp = kept[g0:g0 + 8]
                    at_ps = atpsum.tile([128, 8, 128], BF16, tag="at")
                    at_sb = atsb.tile([128, 8, 128], BF16, tag="atsb")
                    for idx, cch in enumerate(grp):
                        nc.tensor.transpose(at_ps[:, idx, :],
                                            E[:, cch * 128:(cch + 1) * 128], ident)
                    nc.any.tensor_copy(at_sb[:, 0:len(grp), :],
                                       at_ps[:, 0:len(grp), :])
                    for idx, cch in enumerate(grp):
                        nc.tensor.matmul(av_ps, at_sb[:, idx, :], v_sb[:, cch, :],
                                         start=(cch == kept[0]),
                                         stop=(cch == kept[-1]))
                o_sb = outp.tile([128, D], F32, tag="o")
                nc.vector.tensor_scalar_mul(o_sb, av_ps, rden)
                nc.sync.dma_start(out=out_d[t * 128:(t + 1) * 128, :], in_=o_sb)

    nc.compile()
    return nc


def _get_nc():
    if "nc" not in _CACHE:
        _CACHE["nc"] = build_nc()
    return _CACHE["nc"]


def _make_dmask(h):
    m = np.zeros((128, 256), dtype=np.float32)
    upper = np.triu(np.ones((128, 128), dtype=np.float32), k=1)
    if h == 0:
        m[:, 0:128] = upper
        m[:, 128:256] = 1.0
    else:
        m[:, 0:128] = 0.0
        m[:, 128:256] = upper
    return m.astype(NPBF16)


def kernel(x, y, Wq, Wk, Wv):
    nc = _get_nc()
    xb = x.astype(NPBF16)
    yb = y.astype(NPBF16)
    wqb = Wq.astype(NPBF16)
    wkb = Wk.astype(NPBF16)
    wvb = Wv.astype(NPBF16)

    in_maps = []
    yT = {b: np.ascontiguousarray(yb[b].T) for b in range(B)}
    for core in range(NCORES):
        b, h = divmod(core, 2)
        xs = xb[b].reshape(2 * NBLK, 128, C)[h::2].reshape(NLOC, C)
        in_maps.append({
            "xsT": np.ascontiguousarray(xs.T),
            "yT": yT[b],
            "wq": wqb, "wk": wkb, "wv": wvb,
            "dmask": _make_dmask(h),
        })

    if TRACE:
        import tempfile
        tdir = tempfile.mkdtemp(prefix="attn_trace_")
        _CACHE["trace_dir"] = tdir
        res = run_bass_kernel_spmd(nc, in_maps, list(range(NCORES)),
                                   trace=True, tmpdir=tdir)
        _CACHE["exec_time_ns"] = res.exec_time_ns
    else:
        res = run_bass_kernel_spmd(nc, in_maps, list(range(NCORES)))

    out = np.empty((B, N, D), dtype=np.float32)
    for core in range(NCORES):
        b, h = divmod(core, 2)
        out[b].reshape(2 * NBLK, 128, D)[h::2] = \
            res.results[core]["out"].reshape(NBLK, 128, D)
    return out
